# revision 1
# baseline (speedup 1.0000x reference)
"""Causal self-attention (RoPE + QK-RMSNorm, GQA 16q/8kv) Trainium2 Bass kernel.

Sharding: 8 cores = 2 batch x 4 tensor-parallel. Core c handles batch b=c//4 and
q-heads [4*tp, 4*tp+4), kv-heads [2*tp, 2*tp+2) where tp=c%4. Each core returns a
partial (T, C) output = O_heads @ wo[rows of its heads]; host sums the 4 partials
per batch (the "all-reduce after c_proj").

Matmuls run in bf16 (fp32 PSUM accumulation); softmax row-sum normalization and
RMS statistics stay in fp32/fp32r.
"""
import sys
import math

sys.path.insert(0, "/opt/trn_rl_repo")

import numpy as np
import ml_dtypes
import concourse.bacc as bacc
import concourse.mybir as mybir
import concourse.tile as tile
from concourse.bass_utils import run_bass_kernel_spmd

P = 128
T = 2048
C = 2048
KO = C // P          # 16 contraction tiles
D = 128              # head dim
NQ = 4               # q heads per core
NK = 2               # kv heads per core
NF = NQ + NK         # 6 rope/rms feature blocks (4 q + 2 k)
FQ = NQ * D          # 512
FK = NK * D          # 256
TCH = 512            # phase-1 T-chunk
NCHUNK = T // TCH    # 4
SPAN = 512           # attention q-span
NSPAN = T // SPAN    # 4
KB = T // P          # 16 key blocks
SCALE = 1.0 / math.sqrt(D)
EPS = 1.1920929e-07

f32 = mybir.dt.float32
f32r = mybir.dt.float32r
bf16 = mybir.dt.bfloat16

AF = mybir.ActivationFunctionType


def build():
    nc = bacc.Bacc("TRN2", target_bir_lowering=False)
    xT = nc.dram_tensor("xT", (C, T), bf16, kind="ExternalInput")
    wq = nc.dram_tensor("wq", (C, FQ), bf16, kind="ExternalInput")
    wk = nc.dram_tensor("wk", (C, FK), bf16, kind="ExternalInput")
    wv = nc.dram_tensor("wv", (C, FK), bf16, kind="ExternalInput")
    wo = nc.dram_tensor("wo", (FQ, C), bf16, kind="ExternalInput")
    cc = nc.dram_tensor("cc", (P, T), f32, kind="ExternalInput")    # [cos; cos]
    ss = nc.dram_tensor("ss", (P, T), f32, kind="ExternalInput")    # [sin; -sin]
    maskT = nc.dram_tensor("maskT", (P, 4, SPAN), bf16, kind="ExternalInput")
    ident = nc.dram_tensor("ident", (P, P), bf16, kind="ExternalInput")
    y = nc.dram_tensor("y", (T, C), f32, kind="ExternalOutput")

    xT_r = xT.rearrange("(ko p) t -> p ko t", p=P)
    wq_r = wq.rearrange("(ko p) f -> p ko f", p=P)
    wk_r = wk.rearrange("(ko p) f -> p ko f", p=P)
    wv_r = wv.rearrange("(ko p) f -> p ko f", p=P)
    wo_r = wo.rearrange("(ko p) n -> p ko n", p=P)

    with tile.TileContext(nc) as tc:
        with tc.tile_pool(name="persist", bufs=1) as persist:
            # persistent across phases
            qk_rt = persist.tile([P, NF, T], bf16, tag="qk_rt")   # roped+normed qT/kT
            v_sb = persist.tile([P, KB, FK], bf16, tag="v_sb")    # V natural [t-part, kb, feat]
            cc_sb = persist.tile([P, T], f32, tag="cc_sb")
            ss_sb = persist.tile([P, T], f32, tag="ss_sb")
            id_sb = persist.tile([P, P], bf16, tag="id_sb")
            ones_col = persist.tile([P, 1], bf16, tag="ones_col")    # sums lhsT
            ones_row = persist.tile([1, P], f32r, tag="ones_row")    # bcast lhsT
            eps_sb = persist.tile([P, 1], f32, tag="eps_sb")
            zero_sb = persist.tile([1, 1], f32, tag="zero_sb")
            nc.vector.memset(zero_sb[:], 0.0)
            ones_f32 = persist.tile([P, 1], f32, tag="ones_f32")
            ones_row_f32 = persist.tile([1, P], f32, tag="ones_row_f32")
            nc.sync.dma_start(cc_sb[:], cc[:, :])
            nc.sync.dma_start(ss_sb[:], ss[:, :])
            nc.sync.dma_start(id_sb[:], ident[:, :])
            nc.vector.memset(eps_sb[:], EPS)
            nc.vector.memset(ones_f32[:], 1.0)
            nc.vector.memset(ones_row_f32[:], 1.0)
            nc.vector.tensor_copy(ones_col[:], ones_f32[:])
            nc.vector.tensor_copy(ones_row[:], ones_row_f32[:])

            # ------- Phase 1: QKV projections + RoPE + RMS norm + V transpose -------
            with (
                tc.tile_pool(name="ph1w", bufs=1) as wpool,
                tc.tile_pool(name="ph1x", bufs=2) as xpool,
                tc.tile_pool(name="ph1t", bufs=3) as tpool,
                tc.tile_pool(name="ph1ps", bufs=3, space="PSUM") as ps1,
                tc.tile_pool(name="ph1tr", bufs=1, space="PSUM") as pstr,
                tc.tile_pool(name="ph1ms", bufs=2, space="PSUM") as psms,
                tc.tile_pool(name="ph1rb", bufs=2, space="PSUM") as psrb,
            ):
                wq_sb = wpool.tile([P, KO, FQ], bf16, tag="wq_sb")
                wk_sb = wpool.tile([P, KO, FK], bf16, tag="wk_sb")
                wv_sb = wpool.tile([P, KO, FK], bf16, tag="wv_sb")
                nc.sync.dma_start(wq_sb[:], wq_r)
                nc.sync.dma_start(wk_sb[:], wk_r)
                nc.sync.dma_start(wv_sb[:], wv_r)

                for tch in range(NCHUNK):
                    t0 = tch * TCH
                    xt = xpool.tile([P, KO, TCH], bf16, tag="xt")
                    # per-ko DMAs so matmuls can start as slices land
                    for ko in range(KO):
                        nc.sync.dma_start(xt[:, ko, :], xT_r[:, ko, t0 : t0 + TCH])
                    # qT / kT feature blocks (4 q heads + 2 k heads)
                    sqs = []
                    for fb in range(NF):
                        if fb < NQ:
                            w_ap = wq_sb[:, :, fb * D : (fb + 1) * D]
                        else:
                            w_ap = wk_sb[:, :, (fb - NQ) * D : (fb - NQ + 1) * D]
                        pqk = ps1.tile([P, TCH], f32, tag="ps_qkv")
                        for ko in range(KO):
                            nc.tensor.matmul(
                                pqk[:], w_ap[:, ko], xt[:, ko, :],
                                start=(ko == 0), stop=(ko == KO - 1),
                            )
                        # rope: raw chunk + half-swapped chunk (fp32), write bf16
                        raw = tpool.tile([P, TCH], f32, tag="rope_raw")
                        nc.vector.tensor_copy(raw[:], pqk[:])
                        swp = tpool.tile([P, TCH], f32, tag="rope_swp")
                        nc.sync.dma_start(swp[0:64, :], raw[64:128, :])
                        nc.sync.dma_start(swp[64:128, :], raw[0:64, :])
                        tmpa = tpool.tile([P, TCH], f32, tag="rope_tmpa")
                        tmpb = tpool.tile([P, TCH], f32, tag="rope_tmpb")
                        seg = qk_rt[:, fb, t0 : t0 + TCH]
                        nc.vector.tensor_mul(tmpa[:], raw[:], cc_sb[:, t0 : t0 + TCH])
                        nc.vector.tensor_mul(tmpb[:], swp[:], ss_sb[:, t0 : t0 + TCH])
                        nc.vector.tensor_add(seg, tmpa[:], tmpb[:])
                        # RMS stats: sum of squares over head dim (partitions)
                        sq = tpool.tile([P, TCH], bf16, tag="sq")
                        nc.vector.tensor_mul(sq[:], seg, seg)
                        pms = psms.tile([1, TCH], f32, tag="ps_ms")
                        nc.tensor.matmul(pms[:], ones_col[:], sq[:], start=True, stop=True)
                        # rstd = exp(-0.5 * ln(ms/D + eps)) — both on ACT, off the PE path
                        lnms = tpool.tile([1, TCH], f32, tag="lnms")
                        nc.scalar.activation(
                            lnms[:], pms[:], AF.Ln, bias=eps_sb[0:1, :], scale=1.0 / D
                        )
                        rstd = tpool.tile([1, TCH], f32r, tag="rstd")
                        nc.scalar.activation(rstd[:], lnms[:], AF.Exp, scale=-0.5)
                        sqs.append((seg, rstd))
                    # RMS apply pass — bcast matmuls run a full block later so the
                    # ACT chain has drained and the PE never head-of-line blocks
                    for seg, rstd in sqs:
                        pb = psrb.tile([P, TCH], f32, tag="ps_b")
                        nc.tensor.matmul(pb[:], ones_row[:], rstd[:], start=True, stop=True)
                        nc.vector.tensor_mul(seg, seg, pb[:])
                    # vT blocks -> transpose -> V natural
                    for vfb in range(NK):
                        w_ap = wv_sb[:, :, vfb * D : (vfb + 1) * D]
                        pvt = ps1.tile([P, TCH], f32, tag="ps_qkv")
                        for ko in range(KO):
                            nc.tensor.matmul(
                                pvt[:], w_ap[:, ko], xt[:, ko, :],
                                start=(ko == 0), stop=(ko == KO - 1),
                            )
                        vt_sb = tpool.tile([P, TCH], bf16, tag="vt_sb")
                        nc.vector.tensor_copy(vt_sb[:], pvt[:])
                        for tb in range(TCH // P):
                            ptr = pstr.tile([P, P], bf16, tag="ps_tr")
                            nc.tensor.transpose(
                                ptr[:], vt_sb[:, tb * P : (tb + 1) * P], id_sb[:]
                            )
                            nc.vector.tensor_copy(
                                v_sb[:, tch * (TCH // P) + tb, vfb * D : (vfb + 1) * D],
                                ptr[:],
                            )

            # ---------------- Phase 3: attention + Phase 4: output projection ------------
            with (
                tc.tile_pool(name="ph3s", bufs=1) as p3s,
                tc.tile_pool(name="ph3t", bufs=6) as p3,
            ):
                ot_sb = p3s.tile([P, NQ, T], bf16, tag="ot_sb")
                mask_sb = p3s.tile([P, 4, SPAN], bf16, tag="mask_sb")
                wo_sb = p3s.tile([P, NQ, C], bf16, tag="wo_sb")
                nc.sync.dma_start(mask_sb[:], maskT[:, :, :])
                nc.sync.dma_start(wo_sb[:], wo_r)

                with (
                    tc.tile_pool(name="ph3ps", bufs=3, space="PSUM") as ps3,
                    tc.tile_pool(name="ph3ot", bufs=2, space="PSUM") as psot,
                    tc.tile_pool(name="ph3m", bufs=1, space="PSUM") as psm,
                ):
                  for s in range(NSPAN):
                    q0 = s * SPAN
                    nkb = 4 * s + 4
                    for h in range(NQ):
                        j = h // 2
                        ot_ps = psot.tile([P, SPAN], f32, tag="ot_ps")
                        sum_ps = psot.tile([1, SPAN], f32, tag="sum_ps")
                        q_ap = qk_rt[:, h, q0 : q0 + SPAN]
                        for kb in range(nkb):
                            st_ps = ps3.tile([P, SPAN], f32, tag="st_ps")
                            nc.tensor.matmul(
                                st_ps[:],
                                qk_rt[:, NQ + j, kb * P : (kb + 1) * P],
                                q_ap,
                                start=True, stop=True,
                            )
                            pt = p3.tile([P, SPAN], bf16, tag="pt")
                            nc.scalar.activation(pt[:], st_ps[:], AF.Exp, scale=SCALE)
                            if kb >= 4 * s:
                                nc.vector.tensor_mul(
                                    pt[:], pt[:], mask_sb[:, kb - 4 * s, :]
                                )
                            nc.tensor.matmul(
                                ot_ps[:],
                                v_sb[:, kb, j * D : (j + 1) * D],
                                pt[:],
                                start=(kb == 0), stop=(kb == nkb - 1),
                                skip_group_check=True,
                            )
                            nc.tensor.matmul(
                                sum_ps[:],
                                ones_col[:],
                                pt[:],
                                start=(kb == 0), stop=(kb == nkb - 1),
                                skip_group_check=True,
                            )
                        # normalization: 1/sums = exp(-ln(sums)) on ACT, then bcast
                        lns = p3.tile([1, SPAN], f32, tag="lns")
                        nc.scalar.activation(lns[:], sum_ps[:], AF.Ln)
                        rec = p3.tile([1, SPAN], f32r, tag="rec")
                        nc.scalar.activation(rec[:], lns[:], AF.Exp, scale=-1.0)
                        bc_ps = psm.tile([P, SPAN], f32, tag="m512")
                        nc.tensor.matmul(bc_ps[:], ones_row[:], rec[:], start=True, stop=True)
                        bc_sb = p3.tile([P, SPAN], f32, tag="bc_sb")
                        nc.scalar.activation(bc_sb[:], bc_ps[:], AF.Copy)
                        nc.vector.tensor_mul(
                            ot_sb[:, h, q0 : q0 + SPAN], ot_ps[:], bc_sb[:]
                        )

                    # output projection for the T-blocks of this span
                    for tb in range(4 * s, 4 * s + 4):
                        for nch in range(C // 512):
                            yps = psm.tile([P, 512], f32, tag="m512")
                            for h in range(NQ):
                                nc.tensor.matmul(
                                    yps[:],
                                    ot_sb[:, h, tb * P : (tb + 1) * P],
                                    wo_sb[:, h, nch * 512 : (nch + 1) * 512],
                                    start=(h == 0), stop=(h == NQ - 1),
                                )
                            ysb = p3.tile([P, 512], f32, tag="ysb")
                            nc.vector.tensor_copy(ysb[:], yps[:])
                            nc.sync.dma_start(
                                y[tb * P : (tb + 1) * P, nch * 512 : (nch + 1) * 512],
                                ysb[:],
                            )
    nc.compile()
    return nc


_NC_CACHE = None


def _get_nc():
    global _NC_CACHE
    if _NC_CACHE is None:
        _NC_CACHE = build()
    return _NC_CACHE


def _host_inputs(x, cos, sin, wq, wk, wv, wo):
    """Build the 8 per-core input maps."""
    bft = ml_dtypes.bfloat16
    cosT = np.ascontiguousarray(cos[0, :, 0, :].T).astype(np.float32)  # (64, T)
    sinT = np.ascontiguousarray(sin[0, :, 0, :].T).astype(np.float32)
    cc = np.concatenate([cosT, cosT], axis=0)          # (128, T)
    ss = np.concatenate([sinT, -sinT], axis=0)
    # maskT[r][k, q] = 1 if q >= 128*r + k  (within a 512-q span, k-block offset r)
    qidx = np.arange(SPAN)[None, None, :]
    kidx = np.arange(P)[:, None, None]
    ridx = np.arange(4)[None, :, None]
    maskT = (qidx >= P * ridx + kidx).astype(bft)  # (128, 4, 512)
    ident = np.eye(P, dtype=np.float32).astype(bft)

    xTs = [np.ascontiguousarray(x[b].T).astype(bft) for b in range(2)]
    wq16 = wq.astype(bft)
    wk16 = wk.astype(bft)
    wv16 = wv.astype(bft)
    wo16 = wo.astype(bft)
    in_maps = []
    for c in range(8):
        b, tp = divmod(c, 4)
        in_maps.append(
            {
                "xT": xTs[b],
                "wq": np.ascontiguousarray(wq16[:, tp * FQ : (tp + 1) * FQ]),
                "wk": np.ascontiguousarray(wk16[:, tp * FK : (tp + 1) * FK]),
                "wv": np.ascontiguousarray(wv16[:, tp * FK : (tp + 1) * FK]),
                "wo": np.ascontiguousarray(wo16[tp * FQ : (tp + 1) * FQ, :]),
                "cc": cc,
                "ss": ss,
                "maskT": maskT,
                "ident": ident,
            }
        )
    return in_maps


def kernel(x, cos, sin, wq, wk, wv, wo, trace=False):
    x = np.asarray(x, dtype=np.float32)
    cos = np.asarray(cos, dtype=np.float32)
    sin = np.asarray(sin, dtype=np.float32)
    wq = np.asarray(wq, dtype=np.float32)
    wk = np.asarray(wk, dtype=np.float32)
    wv = np.asarray(wv, dtype=np.float32)
    wo = np.asarray(wo, dtype=np.float32)

    nc = _get_nc()
    in_maps = _host_inputs(x, cos, sin, wq, wk, wv, wo)
    res = run_bass_kernel_spmd(nc, in_maps, core_ids=list(range(8)), trace=trace)
    out = np.zeros((2, T, C), dtype=np.float32)
    for c in range(8):
        b = c // 4
        out[b] += res.results[c]["y"]
    if trace:
        return out, res
    return out



# revision 14
# speedup vs baseline: 1.1242x; 1.1242x over previous
"""Causal self-attention (RoPE + QK-RMSNorm, GQA 16q/8kv) Trainium2 Bass kernel.

Sharding: 8 cores = 2 batch x 4 tensor-parallel. Core c handles batch b=c//4 and
q-heads [4*tp, 4*tp+4), kv-heads [2*tp, 2*tp+2) where tp=c%4. Each core returns a
partial (T, C) output = O_heads @ wo[rows of its heads]; host sums the 4 partials
per batch (the "all-reduce after c_proj").

v2 schedule: chunk-projection (C), attention-span (S) and output-projection (P)
phases are interleaved C0 S0 C1 P0 S1 C2 P1 S2 C3 P2 S3 P3 so the PE stream
never drains. Scalar engine runs only Rsqrt / Exp / Copy (2 activation-table
loads total); softmax 1/sum runs on the DVE via reciprocal_approx_fast.
Diagonal attention blocks are restricted to their valid causal q-range.
"""
import sys
import math

sys.path.insert(0, "/opt/trn_rl_repo")

import numpy as np
import ml_dtypes
import concourse.bacc as bacc
import concourse.mybir as mybir
import concourse.tile as tile
from concourse.bass_utils import run_bass_kernel_spmd

P = 128
T = 2048
C = 2048
KO = C // P          # 16 contraction tiles
D = 128              # head dim
NQ = 4               # q heads per core
NK = 2               # kv heads per core
NF = NQ + NK         # 6 rope/rms feature blocks (4 q + 2 k)
FQ = NQ * D          # 512
FK = NK * D          # 256
TCH = 512            # chunk / span size
NCHUNK = T // TCH    # 4
SPAN = 512
KB = T // P          # 16 key blocks
SCALE = 1.0 / math.sqrt(D)
EPS = 1.1920929e-07
DEPTH = 4            # score-ahead software pipeline depth in attention

f32 = mybir.dt.float32
f32r = mybir.dt.float32r
bf16 = mybir.dt.bfloat16

AF = mybir.ActivationFunctionType


def build():
    nc = bacc.Bacc("TRN2", target_bir_lowering=False)
    xT = nc.dram_tensor("xT", (C, T), bf16, kind="ExternalInput")
    wq = nc.dram_tensor("wq", (C, FQ), bf16, kind="ExternalInput")
    wk = nc.dram_tensor("wk", (C, FK), bf16, kind="ExternalInput")
    wv = nc.dram_tensor("wv", (C, FK), bf16, kind="ExternalInput")
    wo = nc.dram_tensor("wo", (FQ, C), bf16, kind="ExternalInput")
    cc = nc.dram_tensor("cc", (P, T), bf16, kind="ExternalInput")    # [cos; cos]
    ss = nc.dram_tensor("ss", (P, T), bf16, kind="ExternalInput")    # [sin; -sin]
    mask = nc.dram_tensor("mask", (P, P), bf16, kind="ExternalInput")  # [k, qq] = qq>=k
    y = nc.dram_tensor("y", (T, C), bf16, kind="ExternalOutput")

    xT_r = xT.rearrange("(ko p) t -> p ko t", p=P)
    wq_r = wq.rearrange("(ko p) f -> p ko f", p=P)
    wk_r = wk.rearrange("(ko p) f -> p ko f", p=P)
    wv_r = wv.rearrange("(ko p) f -> p ko f", p=P)
    wo_r = wo.rearrange("(ko p) n -> p ko n", p=P)

    with tile.TileContext(nc) as tc:
        with (
            tc.tile_pool(name="persist", bufs=1) as persist,
            tc.tile_pool(name="otp", bufs=2) as otp,
            tc.tile_pool(name="xp", bufs=2) as xp,
            tc.tile_pool(name="tpf", bufs=2) as tpf,
            tc.tile_pool(name="tps", bufs=2) as tps,
            tc.tile_pool(name="sqp", bufs=6) as sqp,
            tc.tile_pool(name="rstdp", bufs=6) as rstdp,
            tc.tile_pool(name="tpt", bufs=6) as tpt,
            tc.tile_pool(name="tpy", bufs=10) as tpy,
            tc.tile_pool(name="ps_mm", bufs=4, space="PSUM") as ps_mm,
            tc.tile_pool(name="ps_ot", bufs=2, space="PSUM") as ps_ot,
            tc.tile_pool(name="ps_sum", bufs=2, space="PSUM") as ps_sum,
        ):
            qk_rt = persist.tile([P, NF, T], bf16, tag="qk_rt")   # roped+normed qT/kT
            v_sb = persist.tile([P, KB, FK], bf16, tag="v_sb")    # V natural [t-part, kb, feat]
            cc_sb = persist.tile([P, T], bf16, tag="cc_sb")
            ss_sb = persist.tile([P, T], bf16, tag="ss_sb")
            mask_sb = persist.tile([P, P], bf16, tag="mask_sb")
            ones_col = persist.tile([P, 1], bf16, tag="ones_col")    # sums lhsT
            ones_row = persist.tile([1, P], f32r, tag="ones_row")    # bcast lhsT
            ones_f32 = persist.tile([P, 1], f32, tag="ones_f32")
            ones_row_f32 = persist.tile([1, P], f32, tag="ones_row_f32")
            wq_sb = persist.tile([P, KO, FQ], bf16, tag="wq_sb")
            wk_sb = persist.tile([P, KO, FK], bf16, tag="wk_sb")
            wv_sb = persist.tile([P, KO, FK], bf16, tag="wv_sb")
            wo_sb = persist.tile([P, NQ, C], bf16, tag="wo_sb")

            nc.sync.dma_start(wq_sb[:], wq_r)
            nc.sync.dma_start(wk_sb[:], wk_r)
            nc.sync.dma_start(wv_sb[:], wv_r)
            nc.sync.dma_start(wo_sb[:], wo_r)
            nc.sync.dma_start(cc_sb[:], cc[:, :])
            nc.sync.dma_start(ss_sb[:], ss[:, :])
            nc.sync.dma_start(mask_sb[:], mask[:, :])
            nc.vector.memset(ones_f32[:], 1.0)
            nc.vector.memset(ones_row_f32[:], 1.0)
            nc.vector.tensor_copy(ones_col[:], ones_f32[:])
            nc.vector.tensor_copy(ones_row[:], ones_row_f32[:])

            def emit_chunk(c):
                t0 = c * TCH
                xt = xp.tile([P, KO, TCH], bf16, tag="xt")
                for ko in range(KO):
                    nc.sync.dma_start(xt[:, ko, :], xT_r[:, ko, t0 : t0 + TCH])
                segs = []
                for fb in range(NF):
                    if fb < NQ:
                        w_ap = wq_sb[:, :, fb * D : (fb + 1) * D]
                    else:
                        w_ap = wk_sb[:, :, (fb - NQ) * D : (fb - NQ + 1) * D]
                    pqk = ps_mm.tile([P, TCH], f32, tag="ps_mm")
                    for ko in range(KO):
                        nc.tensor.matmul(
                            pqk[:], w_ap[:, ko], xt[:, ko, :],
                            start=(ko == 0), stop=(ko == KO - 1),
                        )
                    # rope: raw copy (Scalar; table-safe), half-swap via DMA
                    raw = tpf.tile([P, TCH], f32, tag="raw")
                    nc.scalar.activation(raw[:], pqk[:], AF.Copy)
                    swp = tpf.tile([P, TCH], f32, tag="swp")
                    nc.sync.dma_start(swp[0:64, :], raw[64:128, :])
                    nc.sync.dma_start(swp[64:128, :], raw[0:64, :])
                    tmpa = tpf.tile([P, TCH], f32, tag="tmpa")
                    tmpb = tpf.tile([P, TCH], f32, tag="tmpb")
                    seg = qk_rt[:, fb, t0 : t0 + TCH]
                    nc.vector.tensor_mul(tmpa[:], raw[:], cc_sb[:, t0 : t0 + TCH])
                    nc.vector.tensor_mul(tmpb[:], swp[:], ss_sb[:, t0 : t0 + TCH])
                    nc.vector.tensor_add(seg, tmpa[:], tmpb[:])
                    sq = sqp.tile([P, TCH], bf16, tag="sq")
                    nc.vector.tensor_mul(sq[:], seg, seg)
                    segs.append((seg, sq))
                # V in natural layout (out[t-block, d] = sum_ko xt_ko^T @ wv_ko),
                # interleaved with RMS stats so the PE never waits on the
                # DVE/Scalar rstd chains or the 2-buf ps_sum ring.
                rstds = []

                def emit_stat(fb):
                    pms = ps_sum.tile([1, TCH], f32, tag="ps_sum")
                    nc.tensor.matmul(pms[:], ones_col[:], segs[fb][1][:], start=True, stop=True)
                    # rstd = 1/sqrt(ms) = sqrt(D / pms); eps is negligible vs ms
                    pms_sb = tps.tile([1, TCH], f32, tag="pms_sb")
                    nc.vector.tensor_copy(pms_sb[:], pms[:])
                    inv = tps.tile([1, TCH], f32, tag="inv")
                    nc.vector.reciprocal_approx_fast(inv[:], pms_sb[:])
                    rstd = rstdp.tile([1, TCH], f32r, tag="rstd")
                    nc.scalar.activation(rstd[:], inv[:], AF.Sqrt, scale=float(D))
                    rstds.append(rstd)

                def emit_apply(fb):
                    pb = ps_mm.tile([P, TCH], f32, tag="ps_mm")
                    nc.tensor.matmul(pb[:], ones_row[:], rstds[fb][:], start=True, stop=True)
                    seg = segs[fb][0]
                    nc.vector.tensor_mul(seg, seg, pb[:])

                def emit_v(tb):
                    pv = ps_mm.tile([P, TCH], f32, tag="ps_mm")
                    for ko in range(KO):
                        nc.tensor.matmul(
                            pv[:, :FK],
                            xt[:, ko, tb * P : (tb + 1) * P],
                            wv_sb[:, ko, :],
                            start=(ko == 0), stop=(ko == KO - 1),
                        )
                    nc.vector.tensor_copy(
                        v_sb[:, c * (TCH // P) + tb, :], pv[:, :FK]
                    )

                emit_v(0)
                emit_stat(0)
                emit_v(1)
                emit_stat(1)
                emit_v(2)
                emit_stat(2)
                emit_apply(0)
                emit_v(3)
                emit_stat(3)
                emit_apply(1)
                emit_stat(4)
                emit_apply(2)
                emit_stat(5)
                emit_apply(3)
                emit_apply(4)
                emit_apply(5)

            def emit_span(s):
                q0 = s * SPAN
                nkb = 4 * s + 4
                ot_t = otp.tile([P, NQ, SPAN], bf16, tag="ot_t")
                pending = []  # deferred PE-side normalization from previous head

                def emit_norm(h, ot_ps, rec):
                    bc = ps_mm.tile([P, SPAN], f32, tag="ps_mm")
                    nc.tensor.matmul(bc[:], ones_row[:], rec[:], start=True, stop=True)
                    bc_sb = tps.tile([P, SPAN], f32, tag="bc_sb")
                    nc.vector.tensor_copy(bc_sb[:], bc[:])
                    nc.vector.tensor_mul(ot_t[:, h, :], ot_ps[:], bc_sb[:])

                for h in range(NQ):
                    j = h // 2
                    ot_ps = ps_ot.tile([P, SPAN], f32, tag="ot_ps")
                    sum_ps = ps_sum.tile([1, SPAN], f32, tag="ps_sum")
                    queue = []

                    def flush_one():
                        kb, off, vq, pt = queue.pop(0)
                        nc.tensor.matmul(
                            ot_ps[:, off:],
                            v_sb[:, kb, j * D : (j + 1) * D],
                            pt[:, :vq],
                            start=(kb == 0), stop=(kb == nkb - 1),
                            skip_group_check=True,
                        )
                        nc.tensor.matmul(
                            sum_ps[:, off:],
                            ones_col[:],
                            pt[:, :vq],
                            start=(kb == 0), stop=(kb == nkb - 1),
                            skip_group_check=True,
                        )

                    for kb in range(nkb):
                        r = kb - 4 * s           # >=0: diagonal block
                        off = P * r if r > 0 else 0
                        vq = SPAN - off
                        st = ps_mm.tile([P, SPAN], f32, tag="ps_mm")
                        nc.tensor.matmul(
                            st[:, :vq],
                            qk_rt[:, NQ + j, kb * P : (kb + 1) * P],
                            qk_rt[:, h, q0 + off : q0 + SPAN],
                            start=True, stop=True,
                        )
                        pt = tpt.tile([P, SPAN], bf16, tag="pt")
                        nc.scalar.activation(pt[:, :vq], st[:, :vq], AF.Exp, scale=SCALE)
                        if r >= 0:
                            nc.vector.tensor_mul(pt[:, :P], pt[:, :P], mask_sb[:])
                        queue.append((kb, off, vq, pt))
                        if len(queue) > DEPTH:
                            flush_one()
                        if kb == DEPTH - 1 and pending:
                            emit_norm(*pending.pop())
                    while queue:
                        flush_one()
                    # DVE part of softmax normalization; PE part deferred into
                    # the next head's score stream
                    sum_sb = tps.tile([1, SPAN], f32, tag="sum_sb")
                    nc.vector.tensor_copy(sum_sb[:], sum_ps[:])
                    rec = tps.tile([1, SPAN], f32, tag="rec")
                    nc.vector.reciprocal_approx_fast(rec[:], sum_sb[:])
                    rec_r = tps.tile([1, SPAN], f32r, tag="rec_r")
                    nc.vector.tensor_copy(rec_r[:], rec[:])
                    pending.append((h, ot_ps, rec_r))
                if pending:
                    emit_norm(*pending.pop())
                return ot_t

            def emit_proj(c, ot_t):
                for tb in range(4):
                    for nch in range(C // 512):
                        yps = ps_mm.tile([P, 512], f32, tag="ps_mm")
                        for h in range(NQ):
                            nc.tensor.matmul(
                                yps[:],
                                ot_t[:, h, tb * P : (tb + 1) * P],
                                wo_sb[:, h, nch * 512 : (nch + 1) * 512],
                                start=(h == 0), stop=(h == NQ - 1),
                            )
                        ysb = tpy.tile([P, 512], bf16, tag="ysb")
                        nc.vector.tensor_copy(ysb[:], yps[:])
                        nc.sync.dma_start(
                            y[(4 * c + tb) * P : (4 * c + tb + 1) * P,
                              nch * 512 : (nch + 1) * 512],
                            ysb[:],
                        )

            # C0 S0 C1 P0 S1 C2 P1 S2 C3 P2 S3 P3
            emit_chunk(0)
            ot0 = emit_span(0)
            emit_chunk(1)
            emit_proj(0, ot0)
            ot1 = emit_span(1)
            emit_chunk(2)
            emit_proj(1, ot1)
            ot2 = emit_span(2)
            emit_chunk(3)
            emit_proj(2, ot2)
            ot3 = emit_span(3)
            emit_proj(3, ot3)
    nc.compile()
    return nc


_NC_CACHE = None


def _get_nc():
    global _NC_CACHE
    if _NC_CACHE is None:
        _NC_CACHE = build()
    return _NC_CACHE


def _host_inputs(x, cos, sin, wq, wk, wv, wo):
    """Build the 8 per-core input maps."""
    bft = ml_dtypes.bfloat16
    cosT = np.ascontiguousarray(cos[0, :, 0, :].T).astype(np.float32)  # (64, T)
    sinT = np.ascontiguousarray(sin[0, :, 0, :].T).astype(np.float32)
    cc = np.concatenate([cosT, cosT], axis=0).astype(bft)  # (128, T)
    ss = np.concatenate([sinT, -sinT], axis=0).astype(bft)
    # mask[k, qq] = 1 if qq >= k (within the 128-wide diagonal sub-block)
    qq = np.arange(P)[None, :]
    kk = np.arange(P)[:, None]
    mask = (qq >= kk).astype(bft)  # (128, 128)

    xTs = [np.ascontiguousarray(x[b].T).astype(bft) for b in range(2)]
    wq16 = wq.astype(bft)
    wk16 = wk.astype(bft)
    wv16 = wv.astype(bft)
    wo16 = wo.astype(bft)
    in_maps = []
    for c in range(8):
        b, tp = divmod(c, 4)
        in_maps.append(
            {
                "xT": xTs[b],
                "wq": np.ascontiguousarray(wq16[:, tp * FQ : (tp + 1) * FQ]),
                "wk": np.ascontiguousarray(wk16[:, tp * FK : (tp + 1) * FK]),
                "wv": np.ascontiguousarray(wv16[:, tp * FK : (tp + 1) * FK]),
                "wo": np.ascontiguousarray(wo16[tp * FQ : (tp + 1) * FQ, :]),
                "cc": cc,
                "ss": ss,
                "mask": mask,
            }
        )
    return in_maps


def kernel(x, cos, sin, wq, wk, wv, wo, trace=False):
    x = np.asarray(x, dtype=np.float32)
    cos = np.asarray(cos, dtype=np.float32)
    sin = np.asarray(sin, dtype=np.float32)
    wq = np.asarray(wq, dtype=np.float32)
    wk = np.asarray(wk, dtype=np.float32)
    wv = np.asarray(wv, dtype=np.float32)
    wo = np.asarray(wo, dtype=np.float32)

    nc = _get_nc()
    in_maps = _host_inputs(x, cos, sin, wq, wk, wv, wo)
    res = run_bass_kernel_spmd(nc, in_maps, core_ids=list(range(8)), trace=trace)
    out = np.zeros((2, T, C), dtype=np.float32)
    for c in range(8):
        b = c // 4
        out[b] += res.results[c]["y"].astype(np.float32)
    if trace:
        return out, res
    return out


# revision 19
# speedup vs baseline: 1.2366x; 1.1000x over previous
"""Causal self-attention (RoPE + QK-RMSNorm, GQA 16q/8kv) Trainium2 Bass kernel.

Sharding: 8 cores = 2 batch x 4 tensor-parallel. Core c handles batch b=c//4 and
q-heads [4*tp, 4*tp+4), kv-heads [2*tp, 2*tp+2) where tp=c%4. Each core returns a
partial (T, C) output = O_heads @ wo[rows of its heads]; host sums the 4 partials
per batch (the "all-reduce after c_proj").

v3 schedule: chunk-projection (C), attention-span (S) and output-projection (P)
phases are interleaved C0 S0 C1 P0 S1 C2 P1 S2 C3 P2 S3 P3 so the PE stream
never drains. Scalar runs only Sqrt/Exp/Copy (few activation-table loads);
reciprocals run on the DVE; elementwise casts/adds off the critical path run on
the otherwise-idle GpSimd. Latency tails (RMS apply broadcasts, last-head
softmax normalization) are deferred into the next phase's independent PE stream.
Diagonal attention blocks are restricted to their valid causal q-range.
"""
import sys
import math

sys.path.insert(0, "/opt/trn_rl_repo")

import numpy as np
import ml_dtypes
import concourse.bacc as bacc
import concourse.mybir as mybir
import concourse.tile as tile
from concourse.bass_utils import run_bass_kernel_spmd

P = 128
T = 2048
C = 2048
KO = C // P          # 16 contraction tiles
D = 128              # head dim
NQ = 4               # q heads per core
NK = 2               # kv heads per core
NF = NQ + NK         # 6 rope/rms feature blocks (4 q + 2 k)
FQ = NQ * D          # 512
FK = NK * D          # 256
TCH = 512            # chunk / span size
NCHUNK = T // TCH    # 4
SPAN = 512
KB = T // P          # 16 key blocks
SCALE = 1.0 / math.sqrt(D)
DEPTH = 4            # score-ahead software pipeline depth in attention

f32 = mybir.dt.float32
bf16 = mybir.dt.bfloat16

AF = mybir.ActivationFunctionType


def build():
    nc = bacc.Bacc("TRN2", target_bir_lowering=False)
    xT = nc.dram_tensor("xT", (C, T), bf16, kind="ExternalInput")
    wq = nc.dram_tensor("wq", (C, FQ), bf16, kind="ExternalInput")
    wk = nc.dram_tensor("wk", (C, FK), bf16, kind="ExternalInput")
    wv = nc.dram_tensor("wv", (C, FK), bf16, kind="ExternalInput")
    wo = nc.dram_tensor("wo", (FQ, C), bf16, kind="ExternalInput")
    cc = nc.dram_tensor("cc", (P, T), bf16, kind="ExternalInput")    # [cos; cos]
    ss = nc.dram_tensor("ss", (P, T), bf16, kind="ExternalInput")    # [sin; -sin]
    mask = nc.dram_tensor("mask", (P, P), bf16, kind="ExternalInput")  # [k, qq] = qq>=k
    y = nc.dram_tensor("y", (T, C), bf16, kind="ExternalOutput")

    xT_r = xT.rearrange("(ko p) t -> p ko t", p=P)
    wq_r = wq.rearrange("(ko p) f -> p ko f", p=P)
    wk_r = wk.rearrange("(ko p) f -> p ko f", p=P)
    wv_r = wv.rearrange("(ko p) f -> p ko f", p=P)
    wo_r = wo.rearrange("(ko p) n -> p ko n", p=P)

    with tile.TileContext(nc) as tc:
        with (
            tc.tile_pool(name="persist", bufs=1) as persist,
            tc.tile_pool(name="otp", bufs=2) as otp,
            tc.tile_pool(name="xp", bufs=2) as xp,
            tc.tile_pool(name="tpf", bufs=2) as tpf,
            tc.tile_pool(name="tps", bufs=2) as tps,
            tc.tile_pool(name="sqp", bufs=6) as sqp,
            tc.tile_pool(name="rstdp", bufs=6) as rstdp,
            tc.tile_pool(name="tpt", bufs=6) as tpt,
            tc.tile_pool(name="tpy", bufs=10) as tpy,
            tc.tile_pool(name="ps_mm", bufs=4, space="PSUM") as ps_mm,
            tc.tile_pool(name="ps_ot", bufs=2, space="PSUM") as ps_ot,
            tc.tile_pool(name="ps_sum", bufs=2, space="PSUM") as ps_sum,
        ):
            qk_rt = persist.tile([P, NF, T], bf16, tag="qk_rt")   # roped+normed qT/kT
            v_sb = persist.tile([P, KB, FK], bf16, tag="v_sb")    # V natural [t-part, kb, feat]
            cc_sb = persist.tile([P, T], bf16, tag="cc_sb")
            ss_sb = persist.tile([P, T], bf16, tag="ss_sb")
            mask_sb = persist.tile([P, P], bf16, tag="mask_sb")
            ones_col = persist.tile([P, 1], bf16, tag="ones_col")    # sums lhsT
            ones_row = persist.tile([1, P], bf16, tag="ones_row")    # bcast lhsT
            ones_f32 = persist.tile([P, 1], f32, tag="ones_f32")
            ones_row_f32 = persist.tile([1, P], f32, tag="ones_row_f32")
            wq_sb = persist.tile([P, KO, FQ], bf16, tag="wq_sb")
            wk_sb = persist.tile([P, KO, FK], bf16, tag="wk_sb")
            wv_sb = persist.tile([P, KO, FK], bf16, tag="wv_sb")
            wo_sb = persist.tile([P, NQ, C], bf16, tag="wo_sb")

            # split weight DMAs so the first matmuls wait only on their slice
            for fb in range(NQ):
                nc.sync.dma_start(wq_sb[:, :, fb * D : (fb + 1) * D],
                                  wq_r[:, :, fb * D : (fb + 1) * D])
            for fb in range(NK):
                nc.sync.dma_start(wk_sb[:, :, fb * D : (fb + 1) * D],
                                  wk_r[:, :, fb * D : (fb + 1) * D])
            nc.sync.dma_start(wv_sb[:], wv_r)
            nc.sync.dma_start(wo_sb[:], wo_r)
            nc.sync.dma_start(cc_sb[:], cc[:, :])
            nc.sync.dma_start(ss_sb[:], ss[:, :])
            nc.sync.dma_start(mask_sb[:], mask[:, :])
            nc.vector.memset(ones_f32[:], 1.0)
            nc.vector.memset(ones_row_f32[:], 1.0)
            nc.vector.tensor_copy(ones_col[:], ones_f32[:])
            nc.vector.tensor_copy(ones_row[:], ones_row_f32[:])

            def emit_chunk(c, norm_filler=None):
                """Project chunk c -> roped/normalized qT/kT + natural V.
                Returns thunks: deferred RMS-applies for q heads 1..3 (must run
                before span c's head h reads qk_rt[h])."""
                t0 = c * TCH
                xt = xp.tile([P, KO, TCH], bf16, tag="xt")
                for ko in range(KO):
                    nc.sync.dma_start(xt[:, ko, :], xT_r[:, ko, t0 : t0 + TCH])
                segs = [None] * NF

                def emit_fb(fb):
                    if fb < NQ:
                        w_ap = wq_sb[:, :, fb * D : (fb + 1) * D]
                    else:
                        w_ap = wk_sb[:, :, (fb - NQ) * D : (fb - NQ + 1) * D]
                    pqk = ps_mm.tile([P, TCH], f32, tag="ps_mm")
                    for ko in range(KO):
                        nc.tensor.matmul(
                            pqk[:], w_ap[:, ko], xt[:, ko, :],
                            start=(ko == 0), stop=(ko == KO - 1),
                        )
                    # rope: raw copy on Scalar (table-safe), half-swap via DMA
                    raw = tpf.tile([P, TCH], f32, tag="raw")
                    nc.scalar.activation(raw[:], pqk[:], AF.Copy)
                    swp = tpf.tile([P, TCH], f32, tag="swp")
                    nc.sync.dma_start(swp[0:64, :], raw[64:128, :])
                    nc.sync.dma_start(swp[64:128, :], raw[0:64, :])
                    tmpa = tpf.tile([P, TCH], f32, tag="tmpa")
                    tmpb = tpf.tile([P, TCH], f32, tag="tmpb")
                    seg = qk_rt[:, fb, t0 : t0 + TCH]
                    nc.vector.tensor_mul(tmpa[:], pqk[:], cc_sb[:, t0 : t0 + TCH])
                    nc.vector.tensor_mul(tmpb[:], swp[:], ss_sb[:, t0 : t0 + TCH])
                    nc.gpsimd.tensor_add(seg, tmpa[:], tmpb[:])
                    sq = sqp.tile([P, TCH], bf16, tag="sq")
                    nc.vector.tensor_mul(sq[:], seg, seg)
                    segs[fb] = (seg, sq)

                # K features first so their rstd chains finish earliest
                for fb in (4, 5, 0, 1, 2, 3):
                    emit_fb(fb)
                    if fb == 4 and norm_filler is not None:
                        norm_filler()

                rstds = {}

                def emit_stat(fb):
                    pms = ps_sum.tile([1, TCH], f32, tag="ps_sum")
                    nc.tensor.matmul(pms[:], ones_col[:], segs[fb][1][:], start=True, stop=True)
                    # rstd = 1/sqrt(ms) = sqrt(D / pms); eps is negligible vs ms
                    inv = tps.tile([1, TCH], f32, tag="inv")
                    nc.vector.reciprocal_approx_fast(inv[:], pms[:])
                    rstd = rstdp.tile([1, TCH], bf16, tag="rstd")
                    nc.scalar.activation(rstd[:], inv[:], AF.Sqrt, scale=float(D))
                    rstds[fb] = rstd

                def emit_apply(fb):
                    pb = ps_mm.tile([P, TCH], f32, tag="ps_mm")
                    nc.tensor.matmul(pb[:], ones_row[:], rstds[fb][:], start=True, stop=True)
                    seg = segs[fb][0]
                    nc.vector.tensor_mul(seg, seg, pb[:])

                def emit_v(tb):
                    pv = ps_mm.tile([P, TCH], f32, tag="ps_mm")
                    for ko in range(KO):
                        nc.tensor.matmul(
                            pv[:, :FK],
                            xt[:, ko, tb * P : (tb + 1) * P],
                            wv_sb[:, ko, :],
                            start=(ko == 0), stop=(ko == KO - 1),
                        )
                    nc.vector.tensor_copy(
                        v_sb[:, c * (TCH // P) + tb, :], pv[:, :FK]
                    )

                # stats for K heads first (span c's scores need K normalized),
                # V matmuls as PE filler over the rstd latency chains
                emit_v(0)
                emit_stat(4)
                emit_v(1)
                emit_stat(5)
                emit_v(2)
                emit_stat(0)
                emit_apply(4)
                emit_v(3)
                emit_stat(1)
                emit_apply(5)
                emit_stat(2)
                emit_apply(0)
                emit_stat(3)
                deferred = [lambda fb=fb: emit_apply(fb) for fb in (1, 2, 3)]
                return deferred

            def emit_span(s, fillers):
                """Attention for q-span s. `fillers` are independent PE thunks
                sprinkled into the score stream (popped front-first). Returns
                the deferred normalization thunk of the last head."""
                q0 = s * SPAN
                nkb = 4 * s + 4
                ot_t = otp.tile([P, NQ, SPAN], bf16, tag="ot_t")
                pending = []

                def emit_norm(h, ot_ps, rec_r):
                    bc = ps_mm.tile([P, SPAN], f32, tag="ps_mm")
                    nc.tensor.matmul(bc[:], ones_row[:], rec_r[:], start=True, stop=True)
                    bc_sb = tps.tile([P, SPAN], f32, tag="bc_sb")
                    nc.vector.tensor_copy(bc_sb[:], bc[:])
                    nc.vector.tensor_mul(ot_t[:, h, :], ot_ps[:], bc_sb[:])

                for h in range(NQ):
                    j = h // 2
                    ot_ps = ps_ot.tile([P, SPAN], f32, tag="ot_ps")
                    sum_ps = ps_sum.tile([1, SPAN], f32, tag="ps_sum")
                    queue = []

                    def flush_one():
                        kb, off, vq, pt = queue.pop(0)
                        nc.tensor.matmul(
                            ot_ps[:, off:],
                            v_sb[:, kb, j * D : (j + 1) * D],
                            pt[:, :vq],
                            start=(kb == 0), stop=(kb == nkb - 1),
                            skip_group_check=True,
                        )
                        nc.tensor.matmul(
                            sum_ps[:, off:],
                            ones_col[:],
                            pt[:, :vq],
                            start=(kb == 0), stop=(kb == nkb - 1),
                            skip_group_check=True,
                        )

                    for kb in range(nkb):
                        r = kb - 4 * s           # >=0: diagonal block
                        off = P * r if r > 0 else 0
                        vq = SPAN - off
                        st = ps_mm.tile([P, SPAN], f32, tag="ps_mm")
                        nc.tensor.matmul(
                            st[:, :vq],
                            qk_rt[:, NQ + j, kb * P : (kb + 1) * P],
                            qk_rt[:, h, q0 + off : q0 + SPAN],
                            start=True, stop=True,
                        )
                        pt = tpt.tile([P, SPAN], bf16, tag="pt")
                        nc.scalar.activation(pt[:, :vq], st[:, :vq], AF.Exp, scale=SCALE)
                        if r >= 0:
                            nc.vector.tensor_mul(pt[:, :P], pt[:, :P], mask_sb[:])
                        queue.append((kb, off, vq, pt))
                        if fillers:
                            fillers.pop(0)()
                        if len(queue) > DEPTH:
                            flush_one()
                        if kb == DEPTH - 1 and pending:
                            emit_norm(*pending.pop())
                    while queue:
                        flush_one()
                    # DVE part of softmax normalization; the PE broadcast is
                    # deferred into the next head's (or phase's) PE stream
                    rec = tps.tile([1, SPAN], f32, tag="rec")
                    nc.vector.reciprocal_approx_fast(rec[:], sum_ps[:])
                    rec_r = tps.tile([1, SPAN], bf16, tag="rec_r")
                    nc.vector.tensor_copy(rec_r[:], rec[:])
                    pending.append((h, ot_ps, rec_r))
                last = pending.pop()
                return ot_t, (lambda: emit_norm(*last))

            def emit_proj(c, ot_t, fillers):
                for tb in range(4):
                    for nch in range(C // 512):
                        yps = ps_mm.tile([P, 512], f32, tag="ps_mm")
                        for h in range(NQ):
                            nc.tensor.matmul(
                                yps[:],
                                ot_t[:, h, tb * P : (tb + 1) * P],
                                wo_sb[:, h, nch * 512 : (nch + 1) * 512],
                                start=(h == 0), stop=(h == NQ - 1),
                            )
                        if fillers:
                            fillers.pop(0)()
                        ysb = tpy.tile([P, 512], bf16, tag="ysb")
                        nc.scalar.activation(ysb[:], yps[:], AF.Copy)
                        nc.sync.dma_start(
                            y[(4 * c + tb) * P : (4 * c + tb + 1) * P,
                              nch * 512 : (nch + 1) * 512],
                            ysb[:],
                        )

            # C0 S0 C1 P0 S1 C2 P1 S2 C3 P2 S3 P3 with deferred tails
            # sprinkled into the following phase's independent PE stream.
            d0 = emit_chunk(0)
            ot0, n0 = emit_span(0, d0)
            d1 = emit_chunk(1, norm_filler=n0)
            emit_proj(0, ot0, d1)
            ot1, n1 = emit_span(1, [])
            d2 = emit_chunk(2, norm_filler=n1)
            emit_proj(1, ot1, d2)
            ot2, n2 = emit_span(2, [])
            d3 = emit_chunk(3, norm_filler=n2)
            emit_proj(2, ot2, d3)
            ot3, n3 = emit_span(3, [])
            n3()
            emit_proj(3, ot3, [])
    nc.compile()
    return nc


_NC_CACHE = None


def _get_nc():
    global _NC_CACHE
    if _NC_CACHE is None:
        _NC_CACHE = build()
    return _NC_CACHE


def _host_inputs(x, cos, sin, wq, wk, wv, wo):
    """Build the 8 per-core input maps."""
    bft = ml_dtypes.bfloat16
    cosT = np.ascontiguousarray(cos[0, :, 0, :].T).astype(np.float32)  # (64, T)
    sinT = np.ascontiguousarray(sin[0, :, 0, :].T).astype(np.float32)
    cc = np.concatenate([cosT, cosT], axis=0).astype(bft)  # (128, T)
    ss = np.concatenate([sinT, -sinT], axis=0).astype(bft)
    # mask[k, qq] = 1 if qq >= k (within the 128-wide diagonal sub-block)
    qq = np.arange(P)[None, :]
    kk = np.arange(P)[:, None]
    mask = (qq >= kk).astype(bft)  # (128, 128)

    xTs = [np.ascontiguousarray(x[b].T).astype(bft) for b in range(2)]
    wq16 = wq.astype(bft)
    wk16 = wk.astype(bft)
    wv16 = wv.astype(bft)
    wo16 = wo.astype(bft)
    in_maps = []
    for c in range(8):
        b, tp = divmod(c, 4)
        in_maps.append(
            {
                "xT": xTs[b],
                "wq": np.ascontiguousarray(wq16[:, tp * FQ : (tp + 1) * FQ]),
                "wk": np.ascontiguousarray(wk16[:, tp * FK : (tp + 1) * FK]),
                "wv": np.ascontiguousarray(wv16[:, tp * FK : (tp + 1) * FK]),
                "wo": np.ascontiguousarray(wo16[tp * FQ : (tp + 1) * FQ, :]),
                "cc": cc,
                "ss": ss,
                "mask": mask,
            }
        )
    return in_maps


def kernel(x, cos, sin, wq, wk, wv, wo, trace=False):
    x = np.asarray(x, dtype=np.float32)
    cos = np.asarray(cos, dtype=np.float32)
    sin = np.asarray(sin, dtype=np.float32)
    wq = np.asarray(wq, dtype=np.float32)
    wk = np.asarray(wk, dtype=np.float32)
    wv = np.asarray(wv, dtype=np.float32)
    wo = np.asarray(wo, dtype=np.float32)

    nc = _get_nc()
    in_maps = _host_inputs(x, cos, sin, wq, wk, wv, wo)
    res = run_bass_kernel_spmd(nc, in_maps, core_ids=list(range(8)), trace=trace)
    out = np.zeros((2, T, C), dtype=np.float32)
    for c in range(8):
        b = c // 4
        out[b] += res.results[c]["y"].astype(np.float32)
    if trace:
        return out, res
    return out
